# revision 8
# baseline (speedup 1.0000x reference)
"""CeNN front-end Trainium2 kernel.

Reference computation (per batch image u [1,H,W]):
    control = conv3x3_same(u, W_B) + 0                     # [64,H,W]
    x0 = control
    x_{k+1} = alpha*x_k + beta*(conv3x3_same(tanh(x_k), WA_eff) + control + bias)
    (WA_eff = W_A with diagonal center taps clamped >= 1), 16 steps.

Distribution: 8 cores = (batch b in 0..3) x (H half). Each core owns a
272-row slab (256 valid + 16 halo rows toward the other half). Zero
communication: halo contamination advances one row per step and after 16
steps exactly the 16 halo rows are dirty.

Per-core kernel: channel-major layout [64ch -> partitions, rows, 514 cols
(W+2 zero pad)]. Rows are split into two 64-partition blocks (A on
partitions 0:64, B on 64:128) so elementwise work runs 128 wide.

conv3x3 = 9 accumulating matmuls (K=64 cin, M=64 cout, N=512) at per-tap
free offsets, in bf16 (tanh output + beta-scaled weights).  Two more
identity taps add C'' = beta*(control+bias) stored as a bf16 hi+lo
residual pair (fp32-accurate).  Four PE quadrants (tile_position) process
four rows concurrently.  The state update is one fused DVE op per row:
    x' = (x * alpha) + psum        (scalar_tensor_tensor, fp32 exact)

Time is blocked T=2 steps per pass over DRAM ping-pong buffers with
redundant-halo strips (40-row strips, 34 valid).  Pass 0 computes
control from u with a K=10 fp32 im2col matmul (9 shifted u copies + ones
row for bias).
"""

import math

import numpy as np
import ml_dtypes

import concourse.bacc as bacc
import concourse.tile as tile
from concourse import mybir
from concourse.bass_utils import run_bass_kernel_spmd

F32 = mybir.dt.float32
BF16 = mybir.dt.bfloat16
AF = mybir.ActivationFunctionType
ALU = mybir.AluOpType

FULL_CFG = dict(SLAB=272, HS=34, T=2, NSTEPS=16, RC0=17)


def _derive(cfg):
    d = dict(cfg)
    d["R"] = d["HS"] + 2 * d["T"] + 2          # strip tile rows
    assert d["R"] % 2 == 0
    d["RH"] = d["R"] // 2                      # rows per partition block
    assert d["SLAB"] % d["HS"] == 0
    d["NSTRIP"] = d["SLAB"] // d["HS"]
    assert d["SLAB"] % d["RC0"] == 0
    d["NCHUNK0"] = d["SLAB"] // d["RC0"]
    assert d["NSTEPS"] % d["T"] == 0
    d["NPASS"] = d["NSTEPS"] // d["T"]
    d.setdefault("DBG_P0_OUT", d["NPASS"] == 0)
    d["UROWS"] = d["SLAB"] + 2
    return d


def build(cfg):
    """Build the per-core Bass program. Returns compiled nc."""
    g = _derive(cfg)
    SLAB, HS, T, RC0 = g["SLAB"], g["HS"], g["T"], g["RC0"]
    R, RH, NSTRIP, NCHUNK0, NPASS, UROWS = (
        g["R"], g["RH"], g["NSTRIP"], g["NCHUNK0"], g["NPASS"], g["UROWS"])
    WP = 514
    W = 512

    nc = bacc.Bacc("TRN2", target_bir_lowering=False, debug=False,
                   num_devices=8)

    u_in = nc.dram_tensor("u_in", [UROWS, W], F32, kind="ExternalInput")
    wa_in = nc.dram_tensor("wa_in", [64, 11, 64], BF16, kind="ExternalInput")
    wb_in = nc.dram_tensor("wb_in", [10, 64], F32, kind="ExternalInput")
    nbias_in = nc.dram_tensor("nbias_in", [64, 1], F32, kind="ExternalInput")
    alpha_in = nc.dram_tensor("alpha_in", [1, 1], F32, kind="ExternalInput")
    x_out = nc.dram_tensor("x_out", [64, SLAB, W], F32, kind="ExternalOutput")

    Xd = [nc.dram_tensor(f"Xd{i}", [64, SLAB, WP], F32, kind="Internal")
          for i in range(2)]
    Chi_d = nc.dram_tensor("Chi", [64, SLAB, WP], BF16, kind="Internal")
    Clo_d = nc.dram_tensor("Clo", [64, SLAB, WP], BF16, kind="Internal")

    with tile.TileContext(nc) as tc:
        with tc.tile_pool(name="singles", bufs=1) as singles:
            wa_t = singles.tile([128, 11, 64], BF16)
            nc.sync.dma_start(out=wa_t[0:64], in_=wa_in[:, :, :])
            nc.sync.dma_start(out=wa_t[64:128], in_=wa_in[:, :, :])
            wb_t = singles.tile([10, 64], F32)
            nc.sync.dma_start(out=wb_t, in_=wb_in[:, :])
            nbias_t = singles.tile([64, 1], F32)
            nc.sync.dma_start(out=nbias_t, in_=nbias_in[:, :])
            alpha_t = singles.tile([128, 1], F32)
            nc.sync.dma_start(out=alpha_t, in_=alpha_in[:, :].to_broadcast((128, 1)))
            beta_t = singles.tile([128, 1], F32)
            nc.vector.tensor_scalar(out=beta_t, in0=alpha_t, scalar1=-1.0,
                                    scalar2=1.0, op0=ALU.mult, op1=ALU.add)

            # ---------------- pass 0: control -> x0, C_hi, C_lo -------------
            with tc.tile_pool(name="p0u", bufs=2) as p0u, \
                 tc.tile_pool(name="p0ps", bufs=4, space="PSUM") as p0ps, \
                 tc.tile_pool(name="p0st", bufs=2) as p0st:
                for chk in range(NCHUNK0):
                    c0 = RC0 * chk
                    u9 = p0u.tile([10, RC0, W], F32)
                    nc.vector.memset(u9, 0.0)
                    nc.vector.memset(u9[0:1, :, :], 1.0)
                    for t9 in range(9):
                        kh, kw = divmod(t9, 3)
                        # u9[1+t9, t, c] = u_slab[c0+t+kh-1, c+kw-1]
                        c_lo = max(0, 1 - kw)
                        c_hi = min(W, W + 1 - kw)
                        nc.sync.dma_start(
                            out=u9[t9 + 1:t9 + 2, 0:RC0, c_lo:c_hi],
                            in_=u_in[c0 + kh:c0 + kh + RC0,
                                     c_lo + kw - 1:c_hi + kw - 1],
                        )
                    xst = p0st.tile([64, RC0, WP], F32, tag="xst")
                    chst = p0st.tile([64, RC0, WP], BF16, tag="chst")
                    clst = p0st.tile([64, RC0, WP], BF16, tag="clst")
                    for st in (xst, chst, clst):
                        nc.vector.memset(st[:, :, 0:1], 0.0)
                        nc.vector.memset(st[:, :, 513:514], 0.0)
                    for t in range(RC0):
                        pc = p0ps.tile([64, 512], F32)
                        nc.tensor.matmul(pc, wb_t[0:10, :], u9[0:10, t, :],
                                         start=True, stop=True)
                        # x0 = psum - bias
                        nc.scalar.activation(out=xst[:, t, 1:513], in_=pc,
                                             func=AF.Identity,
                                             bias=nbias_t[0:64], scale=1.0)
                        # C_hi = bf16(beta * psum)
                        nc.scalar.activation(out=chst[:, t, 1:513], in_=pc,
                                             func=AF.Copy,
                                             scale=beta_t[0:64])
                        # C_lo = bf16(beta*psum - C_hi)
                        nc.vector.scalar_tensor_tensor(
                            out=clst[:, t, 1:513], in0=pc,
                            scalar=beta_t[0:64], in1=chst[:, t, 1:513],
                            op0=ALU.mult, op1=ALU.subtract)
                    nc.sync.dma_start(out=Xd[0][:, c0:c0 + RC0, :], in_=xst)
                    if g["DBG_P0_OUT"]:
                        nc.sync.dma_start(out=x_out[:, c0:c0 + RC0, :],
                                          in_=xst[:, :, 1:513])
                    nc.sync.dma_start(out=Chi_d[:, c0:c0 + RC0, :], in_=chst)
                    nc.sync.dma_start(out=Clo_d[:, c0:c0 + RC0, :], in_=clst)

            # ---------------- passes 1..NPASS: T steps each ------------------
            # SBUF strip layout: contiguous row blocks. Tile rows [0, RH) on
            # partitions 0:64 (block A), rows [RH, R) on 64:128 (block B).
            # All of an interior row's taps source its own block, so each
            # PSUM accumulation group uses a single tile_position row-group
            # (mixed row-groups in one group crash the device). The two seam
            # rows (RH-1, RH) put their <=3 cross-block taps into a separate
            # single-source PSUM group, folded in with one extra DVE add.
            with tc.tile_pool(name="xs", bufs=2) as xpool, \
                 tc.tile_pool(name="chs", bufs=1) as chpool, \
                 tc.tile_pool(name="cls", bufs=1) as clpool, \
                 tc.tile_pool(name="th", bufs=g["RH"] + 2) as thpool, \
                 tc.tile_pool(name="ps", bufs=2, space="PSUM") as pspool, \
                 tc.tile_pool(name="pf", bufs=2, space="PSUM") as pfpool:
                for p in range(1, NPASS + 1):
                    src_d = Xd[(p - 1) % 2]
                    dst_d = Xd[p % 2]
                    last = (p == NPASS)
                    for si in range(NSTRIP):
                        o0 = HS * si
                        base = o0 - (T + 1)          # slab row of tile row 0
                        sv_lo = max(0, -base)
                        sv_hi = min(R, SLAB - base)
                        xs = xpool.tile([128, RH, WP], F32)
                        ch = chpool.tile([128, RH, WP], BF16)
                        cl = clpool.tile([128, RH, WP], BF16)
                        for blk in range(2):
                            lo, hi = blk * RH, (blk + 1) * RH
                            pr = slice(blk * 64, blk * 64 + 64)
                            ld_lo, ld_hi = max(lo, sv_lo), min(hi, sv_hi)
                            if ld_lo > lo:
                                nc.vector.memset(xs[pr, 0:ld_lo - lo, :], 0.0)
                            if ld_hi < hi:
                                nc.vector.memset(xs[pr, ld_hi - lo:RH, :], 0.0)
                            nc.sync.dma_start(
                                out=xs[pr, ld_lo - lo:ld_hi - lo, :],
                                in_=src_d[:, base + ld_lo:base + ld_hi, :])
                            nc.sync.dma_start(
                                out=ch[pr, ld_lo - lo:ld_hi - lo, :],
                                in_=Chi_d[:, base + ld_lo:base + ld_hi, :])
                            nc.sync.dma_start(
                                out=cl[pr, ld_lo - lo:ld_hi - lo, :],
                                in_=Clo_d[:, base + ld_lo:base + ld_hi, :])
                        for k in range(1, T + 1):
                            up_lo = max(k, sv_lo)
                            up_hi = min(R - k, sv_hi)
                            th = []
                            for j in range(RH):
                                tt = thpool.tile([128, WP], BF16)
                                nc.scalar.activation(out=tt, in_=xs[:, j, :],
                                                     func=AF.Tanh)
                                th.append(tt)

                            def emit_row(trow, ph, ps_tile, pf_tile):
                                """Taps for output tile row `trow` into
                                ps_tile[ph half]; cross-block taps into
                                pf_tile[ph half]. Returns True if foreign
                                taps were emitted."""
                                dblk, dj = divmod(trow, RH)
                                dp = slice(dblk * 64, dblk * 64 + 64)
                                out_ps = ps_tile[ph * 64:ph * 64 + 64, :]
                                main, foreign = [], []
                                for t9 in range(9):
                                    kh, kw = divmod(t9, 3)
                                    srow = trow + kh - 1
                                    sblk, sj = divmod(srow, RH)
                                    (main if sblk == dblk else foreign).append(
                                        (t9, sblk, sj, kw))
                                for i, (t9, sblk, sj, kw) in enumerate(main):
                                    nc.tensor.matmul(
                                        out_ps, wa_t[dp, t9, :],
                                        th[sj][dp, kw:kw + 512],
                                        start=(i == 0), stop=False,
                                        tile_position=(dblk * 64, ph * 64))
                                nc.tensor.matmul(
                                    out_ps, wa_t[dp, 9, :], ch[dp, dj, 1:513],
                                    start=False, stop=False,
                                    tile_position=(dblk * 64, ph * 64))
                                nc.tensor.matmul(
                                    out_ps, wa_t[dp, 10, :], cl[dp, dj, 1:513],
                                    start=False, stop=True,
                                    tile_position=(dblk * 64, ph * 64))
                                if not foreign:
                                    return False
                                sblk = foreign[0][1]
                                sp = slice(sblk * 64, sblk * 64 + 64)
                                out_pf = pf_tile[ph * 64:ph * 64 + 64, :]
                                for i, (t9, _, sj, kw) in enumerate(foreign):
                                    nc.tensor.matmul(
                                        out_pf, wa_t[sp, t9, :],
                                        th[sj][sp, kw:kw + 512],
                                        start=(i == 0),
                                        stop=(i == len(foreign) - 1),
                                        tile_position=(sblk * 64, ph * 64))
                                return True

                            def upd(prow, prange, in1):
                                """x' = x*alpha + psum for pair slot prow over
                                partition range prange."""
                                nc.vector.scalar_tensor_tensor(
                                    out=xs[prange, prow, 1:513],
                                    in0=xs[prange, prow, 1:513],
                                    scalar=alpha_t[prange], in1=in1,
                                    op0=ALU.mult, op1=ALU.add)

                            def fadd(prow, prange, in1):
                                """x' += foreign psum."""
                                nc.vector.scalar_tensor_tensor(
                                    out=xs[prange, prow, 1:513],
                                    in0=xs[prange, prow, 1:513],
                                    scalar=1.0, in1=in1,
                                    op0=ALU.bypass, op1=ALU.add)

                            def act(trow):
                                return up_lo <= trow < up_hi

                            # quad over pairs (j0, j0+1): rows (j, RH+j)
                            for j0 in range(0, RH, 2):
                                j1 = j0 + 1
                                a0, b0 = act(j0), act(RH + j0)
                                a1, b1 = act(j1), act(RH + j1)
                                fa0 = fb0 = fa1 = fb1 = False
                                P0 = P1 = PF = None
                                need_pf = (
                                    (a0 and j0 == RH - 1) or (b0 and j0 == 0)
                                    or (a1 and j1 == RH - 1)
                                    or (b1 and j1 == 0))
                                if need_pf:
                                    PF = pfpool.tile([128, 512], F32)
                                if a0 or b0:
                                    P0 = pspool.tile([128, 512], F32)
                                    if a0:
                                        fa0 = emit_row(j0, 0, P0, PF)
                                    if b0:
                                        fb0 = emit_row(RH + j0, 1, P0, PF)
                                if a1 or b1:
                                    P1 = pspool.tile([128, 512], F32)
                                    if a1:
                                        fa1 = emit_row(j1, 1, P1, PF)
                                    if b1:
                                        fb1 = emit_row(RH + j1, 0, P1, PF)
                                if a0 and b0:
                                    upd(j0, slice(0, 128), P0)
                                else:
                                    if a0:
                                        upd(j0, slice(0, 64), P0[0:64, :])
                                    if b0:
                                        upd(j0, slice(64, 128), P0[64:128, :])
                                if a1:
                                    upd(j1, slice(0, 64), P1[64:128, :])
                                if b1:
                                    upd(j1, slice(64, 128), P1[0:64, :])
                                if fa0:
                                    fadd(j0, slice(0, 64), PF[0:64, :])
                                if fb0:
                                    fadd(j0, slice(64, 128), PF[64:128, :])
                                if fa1:
                                    fadd(j1, slice(0, 64), PF[64:128, :])
                                if fb1:
                                    fadd(j1, slice(64, 128), PF[0:64, :])
                        # store valid rows (tile rows [T+1, R-T-1))
                        st_lo, st_hi = T + 1, R - (T + 1)
                        for blk in range(2):
                            lo, hi = blk * RH, (blk + 1) * RH
                            pr = slice(blk * 64, blk * 64 + 64)
                            s_lo, s_hi = max(lo, st_lo), min(hi, st_hi)
                            if s_lo >= s_hi:
                                continue
                            if last:
                                nc.sync.dma_start(
                                    out=x_out[:, base + s_lo:base + s_hi, :],
                                    in_=xs[pr, s_lo - lo:s_hi - lo, 1:513])
                            else:
                                nc.sync.dma_start(
                                    out=dst_d[:, base + s_lo:base + s_hi, :],
                                    in_=xs[pr, s_lo - lo:s_hi - lo, :])

    nc.compile()
    return nc


def host_prep(u, W_B, W_A, bias, alpha_logit, cfg):
    """Build per-core input maps. Only valid for the full-size problem."""
    g = _derive(cfg)
    SLAB, UROWS = g["SLAB"], g["UROWS"]
    B = u.shape[0]
    H = u.shape[2]
    Wc = 512

    alpha = np.float32(1.0 / (1.0 + np.exp(-np.float64(alpha_logit))))
    beta = np.float32(1.0) - alpha

    WAe = np.array(W_A, dtype=np.float32).copy()
    idx = np.arange(64)
    WAe[idx, idx, 1, 1] = np.maximum(WAe[idx, idx, 1, 1], np.float32(1.0))

    wa_taps = np.zeros((64, 11, 64), dtype=np.float32)
    for t9 in range(9):
        kh, kw = divmod(t9, 3)
        wa_taps[:, t9, :] = (beta * WAe[:, :, kh, kw]).T   # [cin, cout]
    eye = np.eye(64, dtype=np.float32)
    wa_taps[:, 9, :] = eye
    wa_taps[:, 10, :] = eye
    wa_taps = wa_taps.astype(ml_dtypes.bfloat16)

    bias_vec = np.array(bias, dtype=np.float32).reshape(64)
    wb10 = np.zeros((10, 64), dtype=np.float32)
    wb10[0, :] = bias_vec
    for t9 in range(9):
        kh, kw = divmod(t9, 3)
        wb10[t9 + 1, :] = W_B[:, 0, kh, kw]
    nbias = (-bias_vec).reshape(64, 1).astype(np.float32)
    alpha_arr = np.full((1, 1), alpha, dtype=np.float32)

    in_maps = []
    for core in range(8):
        b, h = divmod(core, 2)
        img = np.asarray(u[b, 0], dtype=np.float32)        # [H, 512]
        u_slab = np.zeros((UROWS, Wc), dtype=np.float32)
        if h == 0:
            # slab rows [-1, SLAB+1) = image rows [-1, SLAB+1)
            u_slab[1:UROWS] = img[0:SLAB + 1]
        else:
            off = H - SLAB                                  # 240
            # slab row s = image row s + off; u_in[j] = image j-1+off
            u_slab[0:UROWS - 1] = img[off - 1:H]
        in_maps.append({
            "u_in": u_slab,
            "wa_in": wa_taps,
            "wb_in": wb10,
            "nbias_in": nbias,
            "alpha_in": alpha_arr,
        })
    return in_maps


_NC_CACHE = {}


def _get_nc(cfg_key=None):
    if "nc" not in _NC_CACHE:
        _NC_CACHE["nc"] = build(FULL_CFG)
    return _NC_CACHE["nc"]


def kernel(u, W_B, W_A, bias, alpha_logit, _trace=False):
    u = np.asarray(u, dtype=np.float32)
    B, _, H, Wc = u.shape
    nc = _get_nc()
    in_maps = host_prep(u, W_B, W_A, bias, alpha_logit, FULL_CFG)
    res = run_bass_kernel_spmd(nc, in_maps, core_ids=list(range(8)),
                               trace=_trace)
    SLAB = FULL_CFG["SLAB"]
    VALID = H // 2                                          # 256
    out = np.zeros((B, 64, H, Wc), dtype=np.float32)
    for core in range(8):
        b, h = divmod(core, 2)
        xo = res.results[core]["x_out"]                     # [64, SLAB, 512]
        if h == 0:
            out[b, :, 0:VALID, :] = xo[:, 0:VALID, :]
        else:
            out[b, :, VALID:H, :] = xo[:, SLAB - VALID:SLAB, :]
    kernel._last_results = res
    return out


# revision 12
# speedup vs baseline: 1.9073x; 1.9073x over previous
"""CeNN front-end Trainium2 kernel.

Reference computation (per batch image u [1,H,W]):
    control = conv3x3_same(u, W_B) + 0                     # [64,H,W]
    x0 = control
    x_{k+1} = alpha*x_k + beta*(conv3x3_same(tanh(x_k), WA_eff) + control + bias)
    (WA_eff = W_A with diagonal center taps clamped >= 1), 16 steps.

Distribution: 8 cores = (batch b in 0..3) x (H half). Each core owns a
272-row slab (256 valid + 16 halo rows toward the other half). Zero
communication: halo contamination advances one row per step and after 16
steps exactly the 16 halo rows are dirty.

Per-core kernel: channel-major layout [64ch -> partitions, rows, 514 cols
(W+2 zero pad)]. Rows are split into two 64-partition blocks (A on
partitions 0:64, B on 64:128) so elementwise work runs 128 wide.

conv3x3 = 9 accumulating matmuls (K=64 cin, M=64 cout, N=512) at per-tap
free offsets, in bf16 (tanh output + beta-scaled weights).  Two more
identity taps add C'' = beta*(control+bias) stored as a bf16 hi+lo
residual pair (fp32-accurate).  Four PE quadrants (tile_position) process
four rows concurrently.  The state update is one fused DVE op per row:
    x' = (x * alpha) + psum        (scalar_tensor_tensor, fp32 exact)

Time is blocked T=2 steps per pass over DRAM ping-pong buffers with
redundant-halo strips (40-row strips, 34 valid).  Pass 0 computes
control from u with a K=10 fp32 im2col matmul (9 shifted u copies + ones
row for bias).
"""

import math

import numpy as np
import ml_dtypes

import concourse.bacc as bacc
import concourse.tile as tile
from concourse import mybir
from concourse.bass_utils import run_bass_kernel_spmd

F32 = mybir.dt.float32
BF16 = mybir.dt.bfloat16
AF = mybir.ActivationFunctionType
ALU = mybir.AluOpType

FULL_CFG = dict(SLAB=272, HS=32, T=2, NSTEPS=16, RC0=17)


def _derive(cfg):
    d = dict(cfg)
    d["R"] = d["HS"] + 2 * d["T"] + 2          # strip tile rows
    assert d["R"] % 2 == 0
    d["RH"] = d["R"] // 2                      # rows per partition block
    # strips may be ragged: last strip covers the remainder
    strips = []
    o0 = 0
    while o0 < d["SLAB"]:
        hs = min(d["HS"], d["SLAB"] - o0)
        assert hs % 2 == 0
        strips.append((o0, hs))
        o0 += hs
    d["STRIPS"] = strips
    d["NSTRIP"] = len(strips)
    assert d["SLAB"] % d["RC0"] == 0
    d["NCHUNK0"] = d["SLAB"] // d["RC0"]
    assert d["NSTEPS"] % d["T"] == 0
    d["NPASS"] = d["NSTEPS"] // d["T"]
    d.setdefault("DBG_P0_OUT", d["NPASS"] == 0)
    d["UROWS"] = d["SLAB"] + 2
    return d


def build(cfg):
    """Build the per-core Bass program. Returns compiled nc."""
    g = _derive(cfg)
    SLAB, HS, T, RC0 = g["SLAB"], g["HS"], g["T"], g["RC0"]
    R, RH, NSTRIP, NCHUNK0, NPASS, UROWS = (
        g["R"], g["RH"], g["NSTRIP"], g["NCHUNK0"], g["NPASS"], g["UROWS"])
    WP = 514
    W = 512

    nc = bacc.Bacc("TRN2", target_bir_lowering=False, debug=False,
                   num_devices=8)

    u_in = nc.dram_tensor("u_in", [UROWS, W], F32, kind="ExternalInput")
    wa_in = nc.dram_tensor("wa_in", [64, 11, 64], BF16, kind="ExternalInput")
    wb_in = nc.dram_tensor("wb_in", [10, 64], F32, kind="ExternalInput")
    nbias_in = nc.dram_tensor("nbias_in", [64, 1], F32, kind="ExternalInput")
    alpha_in = nc.dram_tensor("alpha_in", [1, 1], F32, kind="ExternalInput")
    x_out = nc.dram_tensor("x_out", [64, SLAB, W], F32, kind="ExternalOutput")

    Xd = [nc.dram_tensor(f"Xd{i}", [64, SLAB, WP], F32, kind="Internal")
          for i in range(2)]
    Chi_d = nc.dram_tensor("Chi", [64, SLAB, WP], BF16, kind="Internal")
    Clo_d = nc.dram_tensor("Clo", [64, SLAB, WP], BF16, kind="Internal")

    with tile.TileContext(nc) as tc:
        with tc.tile_pool(name="singles", bufs=1) as singles:
            wa_t = singles.tile([128, 11, 64], BF16)
            nc.sync.dma_start(out=wa_t[0:64], in_=wa_in[:, :, :])
            nc.sync.dma_start(out=wa_t[64:128], in_=wa_in[:, :, :])
            wb_t = singles.tile([10, 64], F32)
            nc.sync.dma_start(out=wb_t, in_=wb_in[:, :])
            nbias_t = singles.tile([64, 1], F32)
            nc.sync.dma_start(out=nbias_t, in_=nbias_in[:, :])
            alpha_t = singles.tile([128, 1], F32)
            nc.sync.dma_start(out=alpha_t, in_=alpha_in[:, :].to_broadcast((128, 1)))
            beta_t = singles.tile([128, 1], F32)
            nc.vector.tensor_scalar(out=beta_t, in0=alpha_t, scalar1=-1.0,
                                    scalar2=1.0, op0=ALU.mult, op1=ALU.add)

            # ---------------- pass 0: control -> x0, C_hi, C_lo -------------
            with tc.tile_pool(name="p0u", bufs=2) as p0u, \
                 tc.tile_pool(name="p0ps", bufs=4, space="PSUM") as p0ps, \
                 tc.tile_pool(name="p0st", bufs=2) as p0st:
                for chk in range(NCHUNK0):
                    c0 = RC0 * chk
                    u9 = p0u.tile([10, RC0, W], F32)
                    nc.vector.memset(u9, 0.0)
                    nc.vector.memset(u9[0:1, :, :], 1.0)
                    for t9 in range(9):
                        kh, kw = divmod(t9, 3)
                        # u9[1+t9, t, c] = u_slab[c0+t+kh-1, c+kw-1]
                        c_lo = max(0, 1 - kw)
                        c_hi = min(W, W + 1 - kw)
                        nc.sync.dma_start(
                            out=u9[t9 + 1:t9 + 2, 0:RC0, c_lo:c_hi],
                            in_=u_in[c0 + kh:c0 + kh + RC0,
                                     c_lo + kw - 1:c_hi + kw - 1],
                        )
                    xst = p0st.tile([64, RC0, WP], F32, tag="xst")
                    chst = p0st.tile([64, RC0, WP], BF16, tag="chst")
                    clst = p0st.tile([64, RC0, WP], BF16, tag="clst")
                    for st in (xst, chst, clst):
                        nc.vector.memset(st[:, :, 0:1], 0.0)
                        nc.vector.memset(st[:, :, 513:514], 0.0)
                    for t in range(RC0):
                        pc = p0ps.tile([64, 512], F32)
                        nc.tensor.matmul(pc, wb_t[0:10, :], u9[0:10, t, :],
                                         start=True, stop=True)
                        # x0 = psum - bias
                        nc.scalar.activation(out=xst[:, t, 1:513], in_=pc,
                                             func=AF.Identity,
                                             bias=nbias_t[0:64], scale=1.0)
                        # C_hi = bf16(beta * psum)
                        nc.scalar.activation(out=chst[:, t, 1:513], in_=pc,
                                             func=AF.Copy,
                                             scale=beta_t[0:64])
                        # C_lo = bf16(beta*psum - C_hi)
                        nc.vector.scalar_tensor_tensor(
                            out=clst[:, t, 1:513], in0=pc,
                            scalar=beta_t[0:64], in1=chst[:, t, 1:513],
                            op0=ALU.mult, op1=ALU.subtract)
                    nc.sync.dma_start(out=Xd[0][:, c0:c0 + RC0, :], in_=xst)
                    if g["DBG_P0_OUT"]:
                        nc.sync.dma_start(out=x_out[:, c0:c0 + RC0, :],
                                          in_=xst[:, :, 1:513])
                    nc.sync.dma_start(out=Chi_d[:, c0:c0 + RC0, :], in_=chst)
                    nc.sync.dma_start(out=Clo_d[:, c0:c0 + RC0, :], in_=clst)

            # ---------------- passes 1..NPASS: T steps each ------------------
            # SBUF strip layout: contiguous row blocks. Tile rows [0, RHs) on
            # partitions 0:64 (block A), rows [RHs, Rs) on 64:128 (block B).
            # All of an interior row's taps source its own block, so each
            # PSUM accumulation group uses a single tile_position row-group
            # (mixed row-groups in one group crash the device). The two seam
            # rows (RHs-1, RHs) put their <=3 cross-block taps into a separate
            # single-source PSUM group, folded in with one extra DVE add.
            # Taps are emitted round-robin across the 4 in-flight rows so the
            # in-order PE FIFO interleaves all 4 quadrant streams.
            with tc.tile_pool(name="xs", bufs=2) as xpool, \
                 tc.tile_pool(name="chs", bufs=2) as chpool, \
                 tc.tile_pool(name="cls", bufs=2) as clpool, \
                 tc.tile_pool(name="th", bufs=g["RH"] + 4) as thpool, \
                 tc.tile_pool(name="ps", bufs=2, space="PSUM") as pspool, \
                 tc.tile_pool(name="pf", bufs=2, space="PSUM") as pfpool:
                for p in range(1, NPASS + 1):
                    src_d = Xd[(p - 1) % 2]
                    dst_d = Xd[p % 2]
                    last = (p == NPASS)
                    for (o0, hs) in g["STRIPS"]:
                        Rs = hs + 2 * T + 2
                        RHs = Rs // 2
                        base = o0 - (T + 1)          # slab row of tile row 0
                        sv_lo = max(0, -base)
                        sv_hi = min(Rs, SLAB - base)
                        xs = xpool.tile([128, RHs, WP], F32, tag="xs")
                        ch = chpool.tile([128, RHs, WP], BF16, tag="ch")
                        cl = clpool.tile([128, RHs, WP], BF16, tag="cl")
                        for blk in range(2):
                            lo, hi = blk * RHs, (blk + 1) * RHs
                            pr = slice(blk * 64, blk * 64 + 64)
                            ld_lo, ld_hi = max(lo, sv_lo), min(hi, sv_hi)
                            if ld_lo > lo:
                                nc.vector.memset(xs[pr, 0:ld_lo - lo, :], 0.0)
                            if ld_hi < hi:
                                nc.vector.memset(xs[pr, ld_hi - lo:RHs, :], 0.0)
                            nc.sync.dma_start(
                                out=xs[pr, ld_lo - lo:ld_hi - lo, :],
                                in_=src_d[:, base + ld_lo:base + ld_hi, :])
                            nc.sync.dma_start(
                                out=ch[pr, ld_lo - lo:ld_hi - lo, :],
                                in_=Chi_d[:, base + ld_lo:base + ld_hi, :])
                            nc.sync.dma_start(
                                out=cl[pr, ld_lo - lo:ld_hi - lo, :],
                                in_=Clo_d[:, base + ld_lo:base + ld_hi, :])
                        for k in range(1, T + 1):
                            up_lo = max(k, sv_lo)
                            up_hi = min(Rs - k, sv_hi)
                            th = []
                            for j in range(RHs):
                                tt = thpool.tile([128, WP], BF16)
                                nc.scalar.activation(out=tt, in_=xs[:, j, :],
                                                     func=AF.Tanh)
                                th.append(tt)

                            def row_taps(trow, ph, ps_tile, pf_tile):
                                """Build this row's matmul arg-list (main
                                group, then foreign group). Returns
                                (list of matmul kwargs, used_foreign)."""
                                dblk, dj = divmod(trow, RHs)
                                dp = slice(dblk * 64, dblk * 64 + 64)
                                out_ps = ps_tile[ph * 64:ph * 64 + 64, :]
                                main, foreign = [], []
                                for t9 in range(9):
                                    kh, kw = divmod(t9, 3)
                                    srow = trow + kh - 1
                                    sblk, sj = divmod(srow, RHs)
                                    (main if sblk == dblk else foreign).append(
                                        (t9, sblk, sj, kw))
                                ops = []
                                for i, (t9, sblk, sj, kw) in enumerate(main):
                                    ops.append(dict(
                                        out=out_ps, lhsT=wa_t[dp, t9, :],
                                        rhs=th[sj][dp, kw:kw + 512],
                                        start=(i == 0), stop=False,
                                        tile_position=(dblk * 64, ph * 64)))
                                ops.append(dict(
                                    out=out_ps, lhsT=wa_t[dp, 9, :],
                                    rhs=ch[dp, dj, 1:513],
                                    start=False, stop=False,
                                    tile_position=(dblk * 64, ph * 64)))
                                ops.append(dict(
                                    out=out_ps, lhsT=wa_t[dp, 10, :],
                                    rhs=cl[dp, dj, 1:513],
                                    start=False, stop=True,
                                    tile_position=(dblk * 64, ph * 64)))
                                if foreign:
                                    sblk = foreign[0][1]
                                    sp = slice(sblk * 64, sblk * 64 + 64)
                                    out_pf = pf_tile[ph * 64:ph * 64 + 64, :]
                                    for i, (t9, _, sj, kw) in enumerate(foreign):
                                        ops.append(dict(
                                            out=out_pf, lhsT=wa_t[sp, t9, :],
                                            rhs=th[sj][sp, kw:kw + 512],
                                            start=(i == 0),
                                            stop=(i == len(foreign) - 1),
                                            tile_position=(sblk * 64,
                                                           ph * 64)))
                                return ops, bool(foreign)

                            def upd(prow, prange, in1):
                                nc.vector.scalar_tensor_tensor(
                                    out=xs[prange, prow, 1:513],
                                    in0=xs[prange, prow, 1:513],
                                    scalar=alpha_t[prange], in1=in1,
                                    op0=ALU.mult, op1=ALU.add)

                            def fadd(prow, prange, in1):
                                nc.vector.scalar_tensor_tensor(
                                    out=xs[prange, prow, 1:513],
                                    in0=xs[prange, prow, 1:513],
                                    scalar=1.0, in1=in1,
                                    op0=ALU.bypass, op1=ALU.add)

                            def act(trow):
                                return up_lo <= trow < up_hi

                            for j0 in range(0, RHs, 2):
                                j1 = j0 + 1
                                a0, b0 = act(j0), act(RHs + j0)
                                if j1 < RHs:
                                    a1, b1 = act(j1), act(RHs + j1)
                                else:
                                    a1 = b1 = False
                                P0 = P1 = PF = None
                                need_pf = ((a0 and j0 == RHs - 1)
                                           or (a1 and j1 == RHs - 1)
                                           or (b0 and j0 == 0))
                                if need_pf:
                                    PF = pfpool.tile([128, 512], F32)
                                if a0 or b0:
                                    P0 = pspool.tile([128, 512], F32, tag="P0")
                                if a1 or b1:
                                    P1 = pspool.tile([128, 512], F32, tag="P1")
                                seqs = []
                                frows = []
                                if a0:
                                    s, f = row_taps(j0, 0, P0, PF)
                                    seqs.append(s)
                                    if f:
                                        frows.append((j0, slice(0, 64), 0))
                                if b0:
                                    s, f = row_taps(RHs + j0, 1, P0, PF)
                                    seqs.append(s)
                                    if f:
                                        frows.append((j0, slice(64, 128), 1))
                                if a1:
                                    s, f = row_taps(j1, 1, P1, PF)
                                    seqs.append(s)
                                    if f:
                                        frows.append((j1, slice(0, 64), 1))
                                if b1:
                                    s, f = row_taps(RHs + j1, 0, P1, PF)
                                    seqs.append(s)
                                    if f:
                                        frows.append((j1, slice(64, 128), 0))
                                # round-robin across rows -> 4 quadrant streams
                                nmax = max((len(s) for s in seqs), default=0)
                                for t in range(nmax):
                                    for s in seqs:
                                        if t < len(s):
                                            nc.tensor.matmul(
                                                s[t]["out"], s[t]["lhsT"],
                                                s[t]["rhs"],
                                                start=s[t]["start"],
                                                stop=s[t]["stop"],
                                                skip_group_check=True,
                                                tile_position=s[t][
                                                    "tile_position"])
                                if a0 and b0:
                                    upd(j0, slice(0, 128), P0)
                                else:
                                    if a0:
                                        upd(j0, slice(0, 64), P0[0:64, :])
                                    if b0:
                                        upd(j0, slice(64, 128), P0[64:128, :])
                                if a1:
                                    upd(j1, slice(0, 64), P1[64:128, :])
                                if b1:
                                    upd(j1, slice(64, 128), P1[0:64, :])
                                for (pj, xsl, phh) in frows:
                                    fadd(pj, xsl,
                                         PF[phh * 64:phh * 64 + 64, :])
                        # store valid rows (tile rows [T+1, Rs-T-1))
                        st_lo, st_hi = T + 1, Rs - (T + 1)
                        for blk in range(2):
                            lo, hi = blk * RHs, (blk + 1) * RHs
                            pr = slice(blk * 64, blk * 64 + 64)
                            s_lo, s_hi = max(lo, st_lo), min(hi, st_hi)
                            if s_lo >= s_hi:
                                continue
                            if last:
                                nc.sync.dma_start(
                                    out=x_out[:, base + s_lo:base + s_hi, :],
                                    in_=xs[pr, s_lo - lo:s_hi - lo, 1:513])
                            else:
                                nc.sync.dma_start(
                                    out=dst_d[:, base + s_lo:base + s_hi, :],
                                    in_=xs[pr, s_lo - lo:s_hi - lo, :])

    nc.compile()
    return nc


def host_prep(u, W_B, W_A, bias, alpha_logit, cfg):
    """Build per-core input maps. Only valid for the full-size problem."""
    g = _derive(cfg)
    SLAB, UROWS = g["SLAB"], g["UROWS"]
    B = u.shape[0]
    H = u.shape[2]
    Wc = 512

    alpha = np.float32(1.0 / (1.0 + np.exp(-np.float64(alpha_logit))))
    beta = np.float32(1.0) - alpha

    WAe = np.array(W_A, dtype=np.float32).copy()
    idx = np.arange(64)
    WAe[idx, idx, 1, 1] = np.maximum(WAe[idx, idx, 1, 1], np.float32(1.0))

    wa_taps = np.zeros((64, 11, 64), dtype=np.float32)
    for t9 in range(9):
        kh, kw = divmod(t9, 3)
        wa_taps[:, t9, :] = (beta * WAe[:, :, kh, kw]).T   # [cin, cout]
    eye = np.eye(64, dtype=np.float32)
    wa_taps[:, 9, :] = eye
    wa_taps[:, 10, :] = eye
    wa_taps = wa_taps.astype(ml_dtypes.bfloat16)

    bias_vec = np.array(bias, dtype=np.float32).reshape(64)
    wb10 = np.zeros((10, 64), dtype=np.float32)
    wb10[0, :] = bias_vec
    for t9 in range(9):
        kh, kw = divmod(t9, 3)
        wb10[t9 + 1, :] = W_B[:, 0, kh, kw]
    nbias = (-bias_vec).reshape(64, 1).astype(np.float32)
    alpha_arr = np.full((1, 1), alpha, dtype=np.float32)

    in_maps = []
    for core in range(8):
        b, h = divmod(core, 2)
        img = np.asarray(u[b, 0], dtype=np.float32)        # [H, 512]
        u_slab = np.zeros((UROWS, Wc), dtype=np.float32)
        if h == 0:
            # slab rows [-1, SLAB+1) = image rows [-1, SLAB+1)
            u_slab[1:UROWS] = img[0:SLAB + 1]
        else:
            off = H - SLAB                                  # 240
            # slab row s = image row s + off; u_in[j] = image j-1+off
            u_slab[0:UROWS - 1] = img[off - 1:H]
        in_maps.append({
            "u_in": u_slab,
            "wa_in": wa_taps,
            "wb_in": wb10,
            "nbias_in": nbias,
            "alpha_in": alpha_arr,
        })
    return in_maps


_NC_CACHE = {}


def _get_nc(cfg_key=None):
    if "nc" not in _NC_CACHE:
        _NC_CACHE["nc"] = build(FULL_CFG)
    return _NC_CACHE["nc"]


def kernel(u, W_B, W_A, bias, alpha_logit, _trace=False):
    u = np.asarray(u, dtype=np.float32)
    B, _, H, Wc = u.shape
    nc = _get_nc()
    in_maps = host_prep(u, W_B, W_A, bias, alpha_logit, FULL_CFG)
    res = run_bass_kernel_spmd(nc, in_maps, core_ids=list(range(8)),
                               trace=_trace)
    SLAB = FULL_CFG["SLAB"]
    VALID = H // 2                                          # 256
    out = np.zeros((B, 64, H, Wc), dtype=np.float32)
    for core in range(8):
        b, h = divmod(core, 2)
        xo = res.results[core]["x_out"]                     # [64, SLAB, 512]
        if h == 0:
            out[b, :, 0:VALID, :] = xo[:, 0:VALID, :]
        else:
            out[b, :, VALID:H, :] = xo[:, SLAB - VALID:SLAB, :]
    kernel._last_results = res
    return out
